# revision 19
# baseline (speedup 1.0000x reference)
"""Causal self-attention (B=2, T=2048, C=1024, H=16, D=64) on 8 TRN2 cores.

Sharding: core = b*4 + g handles batch b, heads 4g..4g+3 (data parallel on B,
tensor parallel on heads). Each core computes its 4 heads' contribution to
x @ W_proj; host sums the 4 partial outputs per batch and adds b_proj.

v5 design (pipelined, engine-balanced; all matmul operands bf16):
  Per t-chunk tci (512 cols): qkT = Wqk.T @ x ([128,512] feat tiles -> DVE
  copy to qT/kT), v = x.T @ Wv (t-major -> DVE copy into ones-augmented
  vaug tiles), then RoPE in place on the chunk: half-swap via SBUF->SBUF
  DMA, *sin on Pool, *cos/+ on DVE.  Attention per (q-chunk, head-pair):
  per k-tile: scores 2 matmuls into one [128,1024] PSUM (both heads),
  exp on ACT (the ONLY ACT function -> single table load), causal mask
  multiply on Pool, attn@V with ones-row denominator trick (software
  pipelined one k-tile behind scores).  Normalize: den row -> DVE
  reciprocal_approx_fast -> f32r ones-broadcast matmul -> DVE multiply
  into yT.  Out projection per t-tile -> DVE copy bf16 -> DMA (partial
  sums finished on host in fp32).
  Emission order A0 A1 B0 A2 B1 C0 A3 B2 C1 B3 C2 C3 keeps PE dense;
  weights load on the scalar HWDGE ring, x chunks on the sync ring.
"""
import os
import numpy as np

import concourse.bass as bass
import concourse.mybir as mybir
from concourse import bacc
from concourse.tile import TileContext
from concourse.bass_utils import run_bass_kernel_spmd

B, T, C, H, D = 2, 2048, 1024, 16, 64
HPC = 4          # heads per core
NCORES = 8
TCH = 512        # t-chunk / q-chunk width
NTC = T // TCH   # 4
NTT = T // 128   # 16 t-tiles
NCC = C // 128   # 8 c-chunks
F32 = mybir.dt.float32
F32R = mybir.dt.float32r
BF16 = mybir.dt.bfloat16
MMDT = BF16      # matmul operand dtype (PSUM accum stays fp32)
OUT_DT = BF16    # DRAM output dtype (host accumulates partials in fp32)
AF = mybir.ActivationFunctionType
ALU = mybir.AluOpType

_prog_cache = {}


def _build_program(has_battn: bool):
    nc = bacc.Bacc("TRN2", target_bir_lowering=False, debug=False,
                   num_devices=NCORES)
    # ---- DRAM I/O (per core) ----
    xT_d = nc.dram_tensor("xT", [C, T], MMDT, kind="ExternalInput")
    wqk_d = nc.dram_tensor("wqk", [C, 4 * 128], MMDT, kind="ExternalInput")
    wv_d = nc.dram_tensor("wv", [C, HPC * D], MMDT, kind="ExternalInput")
    wp_d = nc.dram_tensor("wp", [HPC * D, C], MMDT, kind="ExternalInput")
    cos_d = nc.dram_tensor("cos_t", [128, T], MMDT, kind="ExternalInput")
    sin_d = nc.dram_tensor("sin_t", [128, T], MMDT, kind="ExternalInput")
    mask_d = nc.dram_tensor("masks", [4 * 128, TCH], MMDT, kind="ExternalInput")
    ones1_d = nc.dram_tensor("ones1", [1, 64], BF16, kind="ExternalInput")
    bqk_d = nc.dram_tensor("bqk", [4 * 128, 1], F32, kind="ExternalInput")
    vbias_d = nc.dram_tensor("vbias", [128, HPC * D], MMDT, kind="ExternalInput")
    out_d = nc.dram_tensor("out", [T, C], OUT_DT, kind="ExternalOutput")
    dbg = bool(os.environ.get("TRNK_DEBUG"))
    if dbg:
        dq_d = nc.dram_tensor("dbg_qT0", [128, T], F32, kind="ExternalOutput")
        dk_d = nc.dram_tensor("dbg_kT0", [128, T], F32, kind="ExternalOutput")
        dy_d = nc.dram_tensor("dbg_yT0", [128, T], F32, kind="ExternalOutput")
        dv_d = nc.dram_tensor("dbg_va0", [128, HPC * (D + 1)], F32, kind="ExternalOutput")

    with TileContext(nc) as tc:
        with (
            tc.tile_pool(name="wsb", bufs=1) as wsb,      # persistent weights/tables
            tc.tile_pool(name="xsb", bufs=4) as xsb,      # x chunks (all 4 resident)
            tc.tile_pool(name="qk", bufs=1) as qksb,      # persistent qT/kT/yT/vaug
            tc.tile_pool(name="esb", bufs=3) as esb,      # exp tiles
            tc.tile_pool(name="osb", bufs=2) as osb,      # small staging
            tc.tile_pool(name="ps", bufs=2, space="PSUM") as ps,
        ):
            # ---- persistent loads: weights via the scalar HWDGE ring ----
            wqk_sb = wsb.tile([128, NCC, 4 * 128], MMDT, tag="wqk")
            nc.scalar.dma_start(
                out=wqk_sb[:],
                in_=wqk_d[:, :].rearrange("(c p) f -> p c f", p=128))
            wv_sb = wsb.tile([128, NCC, HPC * D], MMDT, tag="wv")
            nc.scalar.dma_start(
                out=wv_sb[:],
                in_=wv_d[:, :].rearrange("(c p) f -> p c f", p=128))
            cos_sb = wsb.tile([128, T], MMDT, tag="cos")
            sin_sb = wsb.tile([128, T], MMDT, tag="sin")
            nc.scalar.dma_start(out=cos_sb[:], in_=cos_d[:, :])
            nc.scalar.dma_start(out=sin_sb[:], in_=sin_d[:, :])
            mask_sb = wsb.tile([128, 4, TCH], MMDT, tag="mask")
            nc.scalar.dma_start(
                out=mask_sb[:],
                in_=mask_d[:, :].rearrange("(m p) c -> p m c", p=128))
            wp_sb = wsb.tile([128, 2, C], MMDT, tag="wp")
            nc.scalar.dma_start(
                out=wp_sb[:],
                in_=wp_d[:, :].rearrange("(k p) n -> p k n", p=128))
            ones1_sb = wsb.tile([1, 64], BF16, tag="ones1")
            nc.scalar.dma_start(out=ones1_sb[:], in_=ones1_d[:, :])
            if has_battn:
                bqk_sb = []
                for ft in range(4):
                    t_ = wsb.tile([128, 1], F32, tag=f"bqk{ft}", name=f"bqk{ft}")
                    nc.scalar.dma_start(
                        out=t_[:], in_=bqk_d[ft * 128:(ft + 1) * 128, :])
                    bqk_sb.append(t_)
                vbias_sb = wsb.tile([128, HPC * D], MMDT, tag="vbias")
                nc.scalar.dma_start(out=vbias_sb[:], in_=vbias_d[:, :])

            # ---- x chunks on the sync ring ----
            xt = []
            for tci in range(NTC):
                t_ = xsb.tile([128, NCC, TCH], MMDT, tag="xt", name=f"xt{tci}")
                nc.sync.dma_start(
                    out=t_[:],
                    in_=xT_d[:, :].rearrange("(c p) t -> p c t", p=128)
                    [:, :, tci * TCH:(tci + 1) * TCH])
                xt.append(t_)

            # persistent activations
            qT = [qksb.tile([128, T], MMDT, tag=f"qT{p}", name=f"qT{p}") for p in range(2)]
            kT = [qksb.tile([128, T], MMDT, tag=f"kT{p}", name=f"kT{p}") for p in range(2)]
            yT = [qksb.tile([128, T], MMDT, tag=f"yT{p}", name=f"yT{p}") for p in range(2)]
            vaug = [qksb.tile([128, HPC * (D + 1)], MMDT, tag=f"va{tt}",
                              name=f"va{tt}")
                    for tt in range(NTT)]
            # ones columns of v_aug via one strided memset per tile (Pool)
            for tt in range(NTT):
                nc.gpsimd.memset(
                    vaug[tt][:].rearrange("p (h e) -> p h e", e=D + 1)
                    [:, :, D:D + 1], 1.0)

            qk_dst = [qT[0], qT[1], kT[0], kT[1]]

            def emit_A(tci):
                ch = slice(tci * TCH, (tci + 1) * TCH)
                # qkT: out [feat 128, TCH] per feat tile
                for ft in range(4):
                    pqk = ps.tile([128, TCH], F32, tag="mm512",
                                  name=f"pqk_{tci}_{ft}")
                    for cc in range(NCC):
                        nc.tensor.matmul(
                            pqk[:],
                            wqk_sb[:, cc:cc + 1, ft * 128:(ft + 1) * 128],
                            xt[tci][:, cc:cc + 1, :],
                            start=(cc == 0), stop=(cc == NCC - 1))
                    dst = qk_dst[ft][:, ch]
                    if has_battn:
                        nc.scalar.activation(dst, pqk[:], AF.Identity,
                                             bias=bqk_sb[ft][:])
                    else:
                        nc.vector.tensor_copy(dst, pqk[:])
                # v: out [t 128, 256] per t-tile
                for j in range(4):
                    tt = tci * 4 + j
                    pv = ps.tile([128, TCH], F32, tag="mm512",
                                 name=f"pv_{tt}")
                    for cc in range(NCC):
                        nc.tensor.matmul(
                            pv[:, 0:HPC * D],
                            xt[tci][:, cc:cc + 1, j * 128:(j + 1) * 128],
                            wv_sb[:, cc:cc + 1, :],
                            start=(cc == 0), stop=(cc == NCC - 1))
                    dst = vaug[tt][:].rearrange(
                        "p (h e) -> p h e", e=D + 1)[:, :, 0:D]
                    src = pv[:, 0:HPC * D].rearrange("p (h e) -> p h e", e=D)
                    if has_battn:
                        nc.vector.scalar_tensor_tensor(
                            dst, src, 0.0,
                            vbias_sb[:].rearrange("p (h e) -> p h e", e=D),
                            ALU.add, ALU.add)
                    else:
                        nc.vector.tensor_copy(dst, src)
                # RoPE in place on this chunk: tmp = swap(X) * sin_signed
                # (4 fused Pool TTs with partition-offset reads), then
                # X = X*cos + tmp on DVE.
                for xi, X in enumerate(qk_dst):
                    tmp = osb.tile([128, TCH], MMDT, tag="rtmp",
                                   name=f"rt_{tci}_{xi}", bufs=2)
                    for b0, b1 in ((0, 32), (32, 0), (64, 96), (96, 64)):
                        nc.gpsimd.tensor_copy(
                            tmp[b0:b0 + 32, :], X[b1:b1 + 32, ch])
                    nc.gpsimd.tensor_tensor(tmp[:], tmp[:], sin_sb[:, ch],
                                            ALU.mult)
                    nc.vector.tensor_tensor(X[:, ch], X[:, ch], cos_sb[:, ch],
                                            ALU.mult)
                    nc.vector.tensor_tensor(X[:, ch], X[:, ch], tmp[:],
                                            ALU.add)

            def emit_B(qc):
                nk = 4 * qc + 4
                qch = slice(qc * TCH, (qc + 1) * TCH)
                for p in range(2):
                    yps = [ps.tile([D + 1, TCH], F32, tag="yps",
                                   name=f"yps_{qc}_{p}_{h}") for h in range(2)]

                    def attn_v(kt, et):
                        for h in range(2):
                            hh = 2 * p + h
                            nc.tensor.matmul(
                                yps[h][:],
                                vaug[kt][:, hh * (D + 1):(hh + 1) * (D + 1)],
                                et[:, h * TCH:(h + 1) * TCH],
                                start=(kt == 0), stop=(kt == nk - 1))

                    prev = None
                    for kt in range(nk):
                        sc = ps.tile([128, 2 * TCH], F32, tag="sc",
                                     name=f"sc_{qc}_{p}_{kt}")
                        for h in range(2):
                            nc.tensor.matmul(
                                sc[:, h * TCH:(h + 1) * TCH],
                                kT[p][h * 64:(h + 1) * 64,
                                      kt * 128:(kt + 1) * 128],
                                qT[p][h * 64:(h + 1) * 64, qch],
                                start=True, stop=True,
                                tile_position=(64 * h, 0))
                        et = esb.tile([128, 2 * TCH], MMDT, tag="et",
                                      name=f"et_{qc}_{p}_{kt}")
                        nc.scalar.activation(et[:], sc[:], AF.Exp, scale=0.125)
                        m = kt - 4 * qc
                        if m >= 0:
                            w = 128 * (m + 1)
                            for h in range(2):
                                nc.gpsimd.tensor_tensor(
                                    et[:, h * TCH:h * TCH + w],
                                    et[:, h * TCH:h * TCH + w],
                                    mask_sb[:, m:m + 1, 0:w], ALU.mult)
                        if prev is not None:
                            attn_v(*prev)
                        prev = (kt, et)
                    attn_v(*prev)
                    # normalize via denominator row: pull yps to SBUF
                    # (recip_approx can't read PSUM), recip the den row,
                    # broadcast via bf16 matmul, multiply into yT.
                    for h in range(2):
                        dr0 = osb.tile([1, TCH], F32, tag=f"dr0{h}",
                                       name=f"dr0_{qc}_{p}_{h}", bufs=2)
                        nc.scalar.copy(dr0[:], yps[h][D:D + 1, :])
                        dr = osb.tile([1, TCH], F32, tag=f"drec{h}",
                                      name=f"drec_{qc}_{p}_{h}", bufs=2)
                        nc.vector.reciprocal_approx_fast(
                            out=dr[:], in_=dr0[:])
                        yv = osb.tile([D, TCH], F32, tag=f"yv{h}",
                                      name=f"yv_{qc}_{p}_{h}", bufs=2)
                        nc.vector.tensor_copy(yv[:], yps[h][0:D, :])
                        drb = osb.tile([1, TCH], BF16, tag=f"drb{h}",
                                       name=f"drb_{qc}_{p}_{h}", bufs=2)
                        nc.gpsimd.tensor_copy(drb[:], dr[:])
                        pb = ps.tile([128, TCH], F32, tag="mm512",
                                     name=f"pb_{qc}_{p}_{h}")
                        nc.tensor.matmul(pb[0:D, :], ones1_sb[:], drb[:],
                                         start=True, stop=True)
                        nc.vector.tensor_tensor(
                            yT[p][h * 64:(h + 1) * 64, qch],
                            yv[:], pb[0:D, :],
                            ALU.mult)

            def emit_C(qc):
                for tt in range(4 * qc, 4 * qc + 4):
                    for nch in range(2):
                        pp = ps.tile([128, TCH], F32, tag="mm512",
                                     name=f"pp_{tt}_{nch}")
                        for kk in range(2):
                            nc.tensor.matmul(
                                pp[:],
                                yT[kk][:, tt * 128:(tt + 1) * 128],
                                wp_sb[:, kk:kk + 1,
                                      nch * TCH:(nch + 1) * TCH],
                                start=(kk == 0), stop=(kk == 1))
                        ot = osb.tile([128, TCH], OUT_DT, tag="ot",
                                      name=f"ot_{tt}_{nch}", bufs=3)
                        nc.vector.tensor_copy(ot[:], pp[:])
                        nc.sync.dma_start(
                            out=out_d[tt * 128:(tt + 1) * 128,
                                      nch * TCH:(nch + 1) * TCH],
                            in_=ot[:])

            emit_A(0)
            emit_A(1)
            emit_B(0)
            emit_A(2)
            emit_B(1)
            emit_C(0)
            emit_A(3)
            emit_B(2)
            emit_C(1)
            emit_B(3)
            emit_C(2)
            emit_C(3)
            if dbg:
                for src_t, dst_t in [(qT[0], dq_d), (kT[0], dk_d),
                                     (yT[0], dy_d)]:
                    dt_ = osb.tile([128, T], F32, tag="dbg",
                                   name=f"dbg_{dst_t.name}", bufs=1)
                    nc.vector.tensor_copy(dt_[:], src_t[:])
                    nc.sync.dma_start(out=dst_t[:, :], in_=dt_[:])
                dvt = osb.tile([128, HPC * (D + 1)], F32, tag="dbgv",
                               name="dbgv", bufs=1)
                nc.vector.tensor_copy(dvt[:], vaug[0][:])
                nc.sync.dma_start(out=dv_d[:, :], in_=dvt[:])

    nc.finalize()
    return nc


def _rope_tables():
    dd = (np.arange(128) % 64) % 32
    fraction = (2.0 * np.arange(32, dtype=np.float32) / 64).astype(np.float32)
    timescale = (np.float32(10000.0) ** fraction).astype(np.float32)
    pos = np.arange(T, dtype=np.float32)
    ang = (pos[None, :] / timescale[dd][:, None]).astype(np.float32)  # [128, T]
    cos_t = np.cos(ang).astype(np.float32)
    sin_t = np.sin(ang).astype(np.float32)
    sgn = np.where((np.arange(128) % 64) < 32, np.float32(-1.0), np.float32(1.0))
    sin_signed = (sin_t * sgn[:, None]).astype(np.float32)
    return cos_t, sin_signed


def _mask_tiles():
    masks = np.zeros((4 * 128, TCH), np.float32)
    r = np.arange(128)[:, None]
    c = np.arange(TCH)[None, :]
    for m in range(4):
        masks[m * 128:(m + 1) * 128] = (c >= 128 * m + r).astype(np.float32)
    return masks


def kernel(x, W_attn, b_attn, W_proj, b_proj):
    x = np.asarray(x, np.float32)
    W_attn = np.asarray(W_attn, np.float32)
    b_attn = np.asarray(b_attn, np.float32)
    W_proj = np.asarray(W_proj, np.float32)
    b_proj = np.asarray(b_proj, np.float32)

    has_battn = bool(np.any(b_attn != 0))
    key = ("v5", has_battn, bool(os.environ.get("TRNK_DEBUG")))
    if key not in _prog_cache:
        _prog_cache[key] = _build_program(has_battn)
    nc = _prog_cache[key]

    import ml_dtypes
    bf = ml_dtypes.bfloat16
    cos_t, sin_signed = _rope_tables()
    masks = _mask_tiles().astype(bf)
    ones1 = np.ones((1, 64), np.float32)

    in_maps = []
    for core in range(NCORES):
        b, g = divmod(core, HPC)
        hs = [HPC * g + i for i in range(HPC)]
        qcols, kcols, vcols = [], [], []
        for i in range(0, HPC, 2):
            qcols += list(range(hs[i] * D, (hs[i] + 1) * D))
            qcols += list(range(hs[i + 1] * D, (hs[i + 1] + 1) * D))
        for i in range(0, HPC, 2):
            kcols += [C + cc for cc in range(hs[i] * D, (hs[i] + 1) * D)]
            kcols += [C + cc for cc in range(hs[i + 1] * D, (hs[i + 1] + 1) * D)]
        vcols = [2 * C + cc for h in hs for cc in range(h * D, (h + 1) * D)]
        rows = [h * D + d for h in hs for d in range(D)]

        in_maps.append({
            "xT": np.ascontiguousarray(x[b].T).astype(bf),
            "wqk": np.ascontiguousarray(W_attn[:, qcols + kcols]).astype(bf),
            "wv": np.ascontiguousarray(W_attn[:, vcols]).astype(bf),
            "wp": np.ascontiguousarray(W_proj[rows, :]).astype(bf),
            "cos_t": cos_t.astype(bf), "sin_t": sin_signed.astype(bf),
            "masks": masks, "ones1": ones1.astype(bf),
            "bqk": np.ascontiguousarray(
                b_attn[qcols + kcols].reshape(-1, 1)),
            "vbias": np.tile(b_attn[vcols], (128, 1)).astype(bf),
        })

    trace = bool(os.environ.get("TRNK_TRACE"))
    if trace:
        try:
            import ntff_shim  # noqa: F401
        except ImportError:
            trace = False
    res = run_bass_kernel_spmd(nc, in_maps, list(range(NCORES)), trace=trace)
    if trace:
        globals()["_last_exec_time_ns"] = res.exec_time_ns
        globals()["_last_trace"] = res.instructions_and_trace
        globals()["_last_profile_json"] = res.profile_json

    globals()["_last_results"] = res.results
    out = np.zeros((B, T, C), np.float32)
    for core in range(NCORES):
        b = core // HPC
        out[b] += np.asarray(res.results[core]["out"], np.float32)
    out += b_proj[None, None, :]
    return out


# revision 20
# speedup vs baseline: 1.8436x; 1.8436x over previous
"""Causal self-attention (B=2, T=2048, C=1024, H=16, D=64) on 8 TRN2 cores.

Sharding: core = b*4 + g handles batch b, heads 4g..4g+3 (data parallel on B,
tensor parallel on heads). Each core computes its 4 heads' contribution to
x @ W_proj; host sums the 4 partial outputs per batch and adds b_proj.

v5 design (pipelined, engine-balanced; all matmul operands bf16):
  Per t-chunk tci (512 cols): qkT = Wqk.T @ x ([128,512] feat tiles -> DVE
  copy to qT/kT), v = x.T @ Wv (t-major -> DVE copy into ones-augmented
  vaug tiles), then RoPE in place on the chunk: half-swap via SBUF->SBUF
  DMA, *sin on Pool, *cos/+ on DVE.  Attention per (q-chunk, head-pair):
  per k-tile: scores 2 matmuls into one [128,1024] PSUM (both heads),
  exp on ACT (the ONLY ACT function -> single table load), causal mask
  multiply on Pool, attn@V with ones-row denominator trick (software
  pipelined one k-tile behind scores).  Normalize: den row -> DVE
  reciprocal_approx_fast -> f32r ones-broadcast matmul -> DVE multiply
  into yT.  Out projection per t-tile -> DVE copy bf16 -> DMA (partial
  sums finished on host in fp32).
  Emission order A0 A1 B0 A2 B1 C0 A3 B2 C1 B3 C2 C3 keeps PE dense;
  weights load on the scalar HWDGE ring, x chunks on the sync ring.
"""
import os
import numpy as np

import concourse.bass as bass
import concourse.mybir as mybir
from concourse import bacc
from concourse.tile import TileContext
from concourse.bass_utils import run_bass_kernel_spmd

B, T, C, H, D = 2, 2048, 1024, 16, 64
HPC = 4          # heads per core
NCORES = 8
TCH = 512        # t-chunk / q-chunk width
NTC = T // TCH   # 4
NTT = T // 128   # 16 t-tiles
NCC = C // 128   # 8 c-chunks
F32 = mybir.dt.float32
F32R = mybir.dt.float32r
BF16 = mybir.dt.bfloat16
MMDT = BF16      # matmul operand dtype (PSUM accum stays fp32)
OUT_DT = BF16    # DRAM output dtype (host accumulates partials in fp32)
AF = mybir.ActivationFunctionType
ALU = mybir.AluOpType

_prog_cache = {}


def _build_program(has_battn: bool):
    nc = bacc.Bacc("TRN2", target_bir_lowering=False, debug=False,
                   num_devices=NCORES)
    # ---- DRAM I/O (per core) ----
    xT_d = nc.dram_tensor("xT", [C, T], MMDT, kind="ExternalInput")
    wqk_d = nc.dram_tensor("wqk", [C, 4 * 128], MMDT, kind="ExternalInput")
    wv_d = nc.dram_tensor("wv", [C, HPC * D], MMDT, kind="ExternalInput")
    wp_d = nc.dram_tensor("wp", [HPC * D, C], MMDT, kind="ExternalInput")
    cos_d = nc.dram_tensor("cos_t", [128, T], MMDT, kind="ExternalInput")
    sin_d = nc.dram_tensor("sin_t", [128, T], MMDT, kind="ExternalInput")
    mask_d = nc.dram_tensor("masks", [4 * 128, TCH], MMDT, kind="ExternalInput")
    ones1_d = nc.dram_tensor("ones1", [1, 64], BF16, kind="ExternalInput")
    perm_d = nc.dram_tensor("perm", [128, 128], BF16, kind="ExternalInput")
    bqk_d = nc.dram_tensor("bqk", [4 * 128, 1], F32, kind="ExternalInput")
    vbias_d = nc.dram_tensor("vbias", [128, HPC * D], MMDT, kind="ExternalInput")
    out_d = nc.dram_tensor("out", [T, C], OUT_DT, kind="ExternalOutput")
    dbg = bool(os.environ.get("TRNK_DEBUG"))
    if dbg:
        dq_d = nc.dram_tensor("dbg_qT0", [128, T], F32, kind="ExternalOutput")
        dk_d = nc.dram_tensor("dbg_kT0", [128, T], F32, kind="ExternalOutput")
        dy_d = nc.dram_tensor("dbg_yT0", [128, T], F32, kind="ExternalOutput")
        dv_d = nc.dram_tensor("dbg_va0", [128, HPC * (D + 1)], F32, kind="ExternalOutput")

    with TileContext(nc) as tc:
        with (
            tc.tile_pool(name="wsb", bufs=1) as wsb,      # persistent weights/tables
            tc.tile_pool(name="xsb", bufs=4) as xsb,      # x chunks (all 4 resident)
            tc.tile_pool(name="qk", bufs=1) as qksb,      # persistent qT/kT/yT/vaug
            tc.tile_pool(name="esb", bufs=3) as esb,      # exp tiles
            tc.tile_pool(name="osb", bufs=2) as osb,      # small staging
            tc.tile_pool(name="ps", bufs=2, space="PSUM") as ps,
        ):
            # ---- persistent loads: weights via the scalar HWDGE ring ----
            wqk_sb = wsb.tile([128, NCC, 4 * 128], MMDT, tag="wqk")
            nc.scalar.dma_start(
                out=wqk_sb[:],
                in_=wqk_d[:, :].rearrange("(c p) f -> p c f", p=128))
            wv_sb = wsb.tile([128, NCC, HPC * D], MMDT, tag="wv")
            nc.scalar.dma_start(
                out=wv_sb[:],
                in_=wv_d[:, :].rearrange("(c p) f -> p c f", p=128))
            cos_sb = wsb.tile([128, T], MMDT, tag="cos")
            sin_sb = wsb.tile([128, T], MMDT, tag="sin")
            nc.scalar.dma_start(out=cos_sb[:], in_=cos_d[:, :])
            nc.scalar.dma_start(out=sin_sb[:], in_=sin_d[:, :])
            mask_sb = wsb.tile([128, 4, TCH], MMDT, tag="mask")
            nc.scalar.dma_start(
                out=mask_sb[:],
                in_=mask_d[:, :].rearrange("(m p) c -> p m c", p=128))
            wp_sb = wsb.tile([128, 2, C], MMDT, tag="wp")
            nc.scalar.dma_start(
                out=wp_sb[:],
                in_=wp_d[:, :].rearrange("(k p) n -> p k n", p=128))
            ones1_sb = wsb.tile([1, 64], BF16, tag="ones1")
            nc.scalar.dma_start(out=ones1_sb[:], in_=ones1_d[:, :])
            perm_sb = wsb.tile([128, 128], BF16, tag="perm")
            nc.scalar.dma_start(out=perm_sb[:], in_=perm_d[:, :])
            if has_battn:
                bqk_sb = []
                for ft in range(4):
                    t_ = wsb.tile([128, 1], F32, tag=f"bqk{ft}", name=f"bqk{ft}")
                    nc.scalar.dma_start(
                        out=t_[:], in_=bqk_d[ft * 128:(ft + 1) * 128, :])
                    bqk_sb.append(t_)
                vbias_sb = wsb.tile([128, HPC * D], MMDT, tag="vbias")
                nc.scalar.dma_start(out=vbias_sb[:], in_=vbias_d[:, :])

            # ---- x chunks on the sync ring ----
            xt = []
            for tci in range(NTC):
                t_ = xsb.tile([128, NCC, TCH], MMDT, tag="xt", name=f"xt{tci}")
                nc.sync.dma_start(
                    out=t_[:],
                    in_=xT_d[:, :].rearrange("(c p) t -> p c t", p=128)
                    [:, :, tci * TCH:(tci + 1) * TCH])
                xt.append(t_)

            # persistent activations
            qT = [qksb.tile([128, T], MMDT, tag=f"qT{p}", name=f"qT{p}") for p in range(2)]
            kT = [qksb.tile([128, T], MMDT, tag=f"kT{p}", name=f"kT{p}") for p in range(2)]
            yT = [qksb.tile([128, T], MMDT, tag=f"yT{p}", name=f"yT{p}") for p in range(2)]
            vaug = [qksb.tile([128, HPC * (D + 1)], MMDT, tag=f"va{tt}",
                              name=f"va{tt}")
                    for tt in range(NTT)]
            # ones columns of v_aug via one strided memset per tile (Pool)
            for tt in range(NTT):
                nc.gpsimd.memset(
                    vaug[tt][:].rearrange("p (h e) -> p h e", e=D + 1)
                    [:, :, D:D + 1], 1.0)

            qk_dst = [qT[0], qT[1], kT[0], kT[1]]

            def emit_A(tci):
                ch = slice(tci * TCH, (tci + 1) * TCH)
                # qkT: out [feat 128, TCH] per feat tile
                for ft in range(4):
                    pqk = ps.tile([128, TCH], F32, tag="mm512",
                                  name=f"pqk_{tci}_{ft}")
                    for cc in range(NCC):
                        nc.tensor.matmul(
                            pqk[:],
                            wqk_sb[:, cc:cc + 1, ft * 128:(ft + 1) * 128],
                            xt[tci][:, cc:cc + 1, :],
                            start=(cc == 0), stop=(cc == NCC - 1))
                    dst = qk_dst[ft][:, ch]
                    if has_battn:
                        nc.scalar.activation(dst, pqk[:], AF.Identity,
                                             bias=bqk_sb[ft][:])
                    else:
                        nc.scalar.copy(dst, pqk[:])
                # v: out [t 128, 256] per t-tile
                for j in range(4):
                    tt = tci * 4 + j
                    pv = ps.tile([128, TCH], F32, tag="mm512",
                                 name=f"pv_{tt}")
                    for cc in range(NCC):
                        nc.tensor.matmul(
                            pv[:, 0:HPC * D],
                            xt[tci][:, cc:cc + 1, j * 128:(j + 1) * 128],
                            wv_sb[:, cc:cc + 1, :],
                            start=(cc == 0), stop=(cc == NCC - 1))
                    dst = vaug[tt][:].rearrange(
                        "p (h e) -> p h e", e=D + 1)[:, :, 0:D]
                    src = pv[:, 0:HPC * D].rearrange("p (h e) -> p h e", e=D)
                    if has_battn:
                        nc.vector.scalar_tensor_tensor(
                            dst, src, 0.0,
                            vbias_sb[:].rearrange("p (h e) -> p h e", e=D),
                            ALU.add, ALU.add)
                    else:
                        nc.vector.tensor_copy(dst, src)
                # RoPE in place on this chunk: Xs = P @ X on PE (signed
                # half-swap), tmp = Xs * sin on DVE, X = X*cos + tmp on DVE.
                for xi, X in enumerate(qk_dst):
                    xs = ps.tile([128, TCH], F32, tag="mm512",
                                 name=f"xs_{tci}_{xi}")
                    nc.tensor.matmul(xs[:], perm_sb[:], X[:, ch],
                                     start=True, stop=True)
                    tmp = osb.tile([128, TCH], MMDT, tag="rtmp",
                                   name=f"rt_{tci}_{xi}", bufs=2)
                    nc.vector.tensor_tensor(tmp[:], xs[:], sin_sb[:, ch],
                                            ALU.mult)
                    nc.vector.tensor_tensor(X[:, ch], X[:, ch], cos_sb[:, ch],
                                            ALU.mult)
                    nc.vector.tensor_tensor(X[:, ch], X[:, ch], tmp[:],
                                            ALU.add)

            def emit_B(qc):
                nk = 4 * qc + 4
                qch = slice(qc * TCH, (qc + 1) * TCH)
                for p in range(2):
                    yps = [ps.tile([D + 1, TCH], F32, tag="yps",
                                   name=f"yps_{qc}_{p}_{h}") for h in range(2)]

                    def attn_v(kt, et):
                        for h in range(2):
                            hh = 2 * p + h
                            nc.tensor.matmul(
                                yps[h][:],
                                vaug[kt][:, hh * (D + 1):(hh + 1) * (D + 1)],
                                et[:, h * TCH:(h + 1) * TCH],
                                start=(kt == 0), stop=(kt == nk - 1))

                    prev = None
                    for kt in range(nk):
                        sc = ps.tile([128, 2 * TCH], F32, tag="sc",
                                     name=f"sc_{qc}_{p}_{kt}")
                        for h in range(2):
                            nc.tensor.matmul(
                                sc[:, h * TCH:(h + 1) * TCH],
                                kT[p][h * 64:(h + 1) * 64,
                                      kt * 128:(kt + 1) * 128],
                                qT[p][h * 64:(h + 1) * 64, qch],
                                start=True, stop=True,
                                tile_position=(64 * h, 0))
                        et = esb.tile([128, 2 * TCH], MMDT, tag="et",
                                      name=f"et_{qc}_{p}_{kt}")
                        nc.scalar.activation(et[:], sc[:], AF.Exp, scale=0.125)
                        m = kt - 4 * qc
                        if m >= 0:
                            w = 128 * (m + 1)
                            for h in range(2):
                                nc.vector.tensor_tensor(
                                    et[:, h * TCH:h * TCH + w],
                                    et[:, h * TCH:h * TCH + w],
                                    mask_sb[:, m:m + 1, 0:w], ALU.mult)
                        if prev is not None:
                            attn_v(*prev)
                        prev = (kt, et)
                    attn_v(*prev)
                    # normalize via denominator row: pull yps to SBUF
                    # (recip_approx can't read PSUM), recip the den row,
                    # broadcast via bf16 matmul, multiply into yT.
                    for h in range(2):
                        dr0 = osb.tile([1, TCH], F32, tag=f"dr0{h}",
                                       name=f"dr0_{qc}_{p}_{h}", bufs=2)
                        nc.scalar.copy(dr0[:], yps[h][D:D + 1, :])
                        dr = osb.tile([1, TCH], F32, tag=f"drec{h}",
                                      name=f"drec_{qc}_{p}_{h}", bufs=2)
                        nc.vector.reciprocal_approx_fast(
                            out=dr[:], in_=dr0[:])
                        yv = osb.tile([D, TCH], F32, tag=f"yv{h}",
                                      name=f"yv_{qc}_{p}_{h}", bufs=2)
                        nc.vector.tensor_copy(yv[:], yps[h][0:D, :])
                        drb = osb.tile([1, TCH], BF16, tag=f"drb{h}",
                                       name=f"drb_{qc}_{p}_{h}", bufs=2)
                        nc.vector.tensor_copy(drb[:], dr[:])
                        pb = ps.tile([128, TCH], F32, tag="mm512",
                                     name=f"pb_{qc}_{p}_{h}")
                        nc.tensor.matmul(pb[0:D, :], ones1_sb[:], drb[:],
                                         start=True, stop=True)
                        nc.vector.tensor_tensor(
                            yT[p][h * 64:(h + 1) * 64, qch],
                            yv[:], pb[0:D, :],
                            ALU.mult)

            def emit_C(qc):
                for tt in range(4 * qc, 4 * qc + 4):
                    for nch in range(2):
                        pp = ps.tile([128, TCH], F32, tag="mm512",
                                     name=f"pp_{tt}_{nch}")
                        for kk in range(2):
                            nc.tensor.matmul(
                                pp[:],
                                yT[kk][:, tt * 128:(tt + 1) * 128],
                                wp_sb[:, kk:kk + 1,
                                      nch * TCH:(nch + 1) * TCH],
                                start=(kk == 0), stop=(kk == 1))
                        ot = osb.tile([128, TCH], OUT_DT, tag="ot",
                                      name=f"ot_{tt}_{nch}", bufs=3)
                        if nch == 0:
                            nc.scalar.copy(ot[:], pp[:])
                        else:
                            nc.vector.tensor_copy(ot[:], pp[:])
                        nc.sync.dma_start(
                            out=out_d[tt * 128:(tt + 1) * 128,
                                      nch * TCH:(nch + 1) * TCH],
                            in_=ot[:])

            emit_A(0)
            emit_A(1)
            emit_B(0)
            emit_A(2)
            emit_B(1)
            emit_C(0)
            emit_A(3)
            emit_B(2)
            emit_C(1)
            emit_B(3)
            emit_C(2)
            emit_C(3)
            if dbg:
                for src_t, dst_t in [(qT[0], dq_d), (kT[0], dk_d),
                                     (yT[0], dy_d)]:
                    dt_ = osb.tile([128, T], F32, tag="dbg",
                                   name=f"dbg_{dst_t.name}", bufs=1)
                    nc.vector.tensor_copy(dt_[:], src_t[:])
                    nc.sync.dma_start(out=dst_t[:, :], in_=dt_[:])
                dvt = osb.tile([128, HPC * (D + 1)], F32, tag="dbgv",
                               name="dbgv", bufs=1)
                nc.vector.tensor_copy(dvt[:], vaug[0][:])
                nc.sync.dma_start(out=dv_d[:, :], in_=dvt[:])

    nc.finalize()
    return nc


def _rope_tables():
    dd = (np.arange(128) % 64) % 32
    fraction = (2.0 * np.arange(32, dtype=np.float32) / 64).astype(np.float32)
    timescale = (np.float32(10000.0) ** fraction).astype(np.float32)
    pos = np.arange(T, dtype=np.float32)
    ang = (pos[None, :] / timescale[dd][:, None]).astype(np.float32)  # [128, T]
    cos_t = np.cos(ang).astype(np.float32)
    sin_t = np.sin(ang).astype(np.float32)
    return cos_t, sin_t


def _mask_tiles():
    masks = np.zeros((4 * 128, TCH), np.float32)
    r = np.arange(128)[:, None]
    c = np.arange(TCH)[None, :]
    for m in range(4):
        masks[m * 128:(m + 1) * 128] = (c >= 128 * m + r).astype(np.float32)
    return masks


def kernel(x, W_attn, b_attn, W_proj, b_proj):
    x = np.asarray(x, np.float32)
    W_attn = np.asarray(W_attn, np.float32)
    b_attn = np.asarray(b_attn, np.float32)
    W_proj = np.asarray(W_proj, np.float32)
    b_proj = np.asarray(b_proj, np.float32)

    has_battn = bool(np.any(b_attn != 0))
    key = ("v6", has_battn, bool(os.environ.get("TRNK_DEBUG")))
    if key not in _prog_cache:
        _prog_cache[key] = _build_program(has_battn)
    nc = _prog_cache[key]

    import ml_dtypes
    bf = ml_dtypes.bfloat16
    cos_t, sin_t = _rope_tables()
    masks = _mask_tiles().astype(bf)
    ones1 = np.ones((1, 64), np.float32)
    perm = np.zeros((128, 128), np.float32)
    for hh in range(2):
        for j in range(32):
            perm[64 * hh + j + 32, 64 * hh + j] = -1.0   # Xs[j] = -X[j+32]
            perm[64 * hh + j, 64 * hh + j + 32] = 1.0    # Xs[j+32] = X[j]


    in_maps = []
    for core in range(NCORES):
        b, g = divmod(core, HPC)
        hs = [HPC * g + i for i in range(HPC)]
        qcols, kcols, vcols = [], [], []
        for i in range(0, HPC, 2):
            qcols += list(range(hs[i] * D, (hs[i] + 1) * D))
            qcols += list(range(hs[i + 1] * D, (hs[i + 1] + 1) * D))
        for i in range(0, HPC, 2):
            kcols += [C + cc for cc in range(hs[i] * D, (hs[i] + 1) * D)]
            kcols += [C + cc for cc in range(hs[i + 1] * D, (hs[i + 1] + 1) * D)]
        vcols = [2 * C + cc for h in hs for cc in range(h * D, (h + 1) * D)]
        rows = [h * D + d for h in hs for d in range(D)]

        in_maps.append({
            "xT": np.ascontiguousarray(x[b].T).astype(bf),
            "wqk": np.ascontiguousarray(W_attn[:, qcols + kcols]).astype(bf),
            "wv": np.ascontiguousarray(W_attn[:, vcols]).astype(bf),
            "wp": np.ascontiguousarray(W_proj[rows, :]).astype(bf),
            "cos_t": cos_t.astype(bf), "sin_t": sin_t.astype(bf),
            "masks": masks, "ones1": ones1.astype(bf), "perm": perm.astype(bf),
            "bqk": np.ascontiguousarray(
                b_attn[qcols + kcols].reshape(-1, 1)),
            "vbias": np.tile(b_attn[vcols], (128, 1)).astype(bf),
        })

    trace = bool(os.environ.get("TRNK_TRACE"))
    if trace:
        try:
            import ntff_shim  # noqa: F401
        except ImportError:
            trace = False
    res = run_bass_kernel_spmd(nc, in_maps, list(range(NCORES)), trace=trace)
    if trace:
        globals()["_last_exec_time_ns"] = res.exec_time_ns
        globals()["_last_trace"] = res.instructions_and_trace
        globals()["_last_profile_json"] = res.profile_json

    globals()["_last_results"] = res.results
    out = np.zeros((B, T, C), np.float32)
    for core in range(NCORES):
        b = core // HPC
        out[b] += np.asarray(res.results[core]["out"], np.float32)
    out += b_proj[None, None, :]
    return out
